# revision 26
# baseline (speedup 1.0000x reference)
"""Trainium2 Bass kernel for nn_Encoder_38259568672815 (ViT-style encoder).

Strategy: data-parallel over batch (16 images -> 8 cores x 2 images).
On-chip layout: feature-major residual stream [D on partitions, tokens free]
in bf16. Software-pipelined over 12 halfsteps (layer, image): attention of
halfstep k+1 (scores+exp on Scalar) overlaps the FFN of halfstep k (PE), so
the Tensor engine stays dense and at full clock.

Self-contained: hardcodes all shapes; host work is limited to layout
permutations (im2col, weight casts, final transpose) and sharding.
"""
from contextlib import ExitStack

import numpy as np
import ml_dtypes

import concourse.bass as bass
import concourse.tile as tile
import concourse.mybir as mybir
from concourse import bacc
from concourse.masks import make_identity
from concourse.bass_utils import run_bass_kernel_spmd

F32 = mybir.dt.float32
BF16 = mybir.dt.bfloat16
AF = mybir.ActivationFunctionType
ADD = mybir.AluOpType.add

B, C, IMG, P = 16, 3, 384, 16
D, NH, DK, L, FF = 768, 12, 64, 6, 3072
S = (IMG // P) ** 2          # 576 tokens per image
NI = 2                       # images per core
T = NI * S                   # 1152 token columns per core
DT = D // 128                # 6 d-tiles
FT = FF // 128               # 24 f-tiles
ST = (S + 127) // 128        # 5 token tiles per image (last = 64)
QCH = 2                      # q chunks of 288 per image
QW = S // QCH                # 288
NG = NH // 2 * QCH           # 12 (head-pair, q-chunk) groups per halfstep
NK = NI * L                  # 12 halfsteps
NCORES = 8


def _stiles():
    return [(kt, kt * 128, min(128, S - kt * 128)) for kt in range(ST)]


def build_kernel(n_layers=L):
    nc = bacc.Bacc()
    nk = NI * n_layers

    xp = nc.dram_tensor("xp", [NI, D, S], BF16, kind="ExternalInput")
    wck = nc.dram_tensor("wck", [D, D], BF16, kind="ExternalInput")
    cb = nc.dram_tensor("cb", [D], F32, kind="ExternalInput")
    pef = nc.dram_tensor("pef", [D, S], F32, kind="ExternalInput")
    wq = nc.dram_tensor("wq", [L, D, D], BF16, kind="ExternalInput")
    wk = nc.dram_tensor("wk", [L, D, D], BF16, kind="ExternalInput")
    wv = nc.dram_tensor("wv", [L, D, D], BF16, kind="ExternalInput")
    wh = nc.dram_tensor("wh", [L, D, D], BF16, kind="ExternalInput")
    whb = nc.dram_tensor("whb", [L, D], F32, kind="ExternalInput")
    ln2s = nc.dram_tensor("ln2s", [L, D], F32, kind="ExternalInput")
    ln2b = nc.dram_tensor("ln2b", [L, D], F32, kind="ExternalInput")
    w1 = nc.dram_tensor("w1", [L, FT, 128, DT, 128], BF16, kind="ExternalInput")
    b1 = nc.dram_tensor("b1", [L, FF], F32, kind="ExternalInput")
    w2 = nc.dram_tensor("w2", [L, FF, D], BF16, kind="ExternalInput")
    b2 = nc.dram_tensor("b2", [L, D], F32, kind="ExternalInput")
    lnfs = nc.dram_tensor("lnfs", [D], F32, kind="ExternalInput")
    lnfb = nc.dram_tensor("lnfb", [D], F32, kind="ExternalInput")
    out = nc.dram_tensor("out", [NI, D, S], F32, kind="ExternalOutput")
    scratch = nc.dram_tensor("scratch", [NI, S, D], BF16)

    with tile.TileContext(nc) as tc, ExitStack() as ctx, \
            nc.allow_low_precision(reason="bf16 residual stream by design"):
        xpool = ctx.enter_context(tc.tile_pool(name="x", bufs=1))
        x1pool = ctx.enter_context(tc.tile_pool(name="x1", bufs=1))
        consts = ctx.enter_context(tc.tile_pool(name="consts", bufs=1))
        biasp = ctx.enter_context(tc.tile_pool(name="biasp", bufs=2))

        ones_col = consts.tile([128, 1], BF16)
        nc.vector.memset(ones_col[:], 1.0)
        ones_f = consts.tile([128, 1], F32)
        nc.vector.memset(ones_f[:], 1.0)
        ones64 = consts.tile([128, 64], BF16)
        nc.vector.memset(ones64[:], 1.0)
        ones_row = consts.tile([1, 128], F32)
        nc.vector.memset(ones_row[:], 1.0)
        eps2 = consts.tile([1, 1], F32)
        nc.vector.memset(eps2[:], 1e-6)
        epsf = consts.tile([1, 1], F32)
        nc.vector.memset(epsf[:], 1e-12)
        ident = consts.tile([128, 128], BF16)
        make_identity(nc, ident[:])

        resid = [xpool.tile([128, DT, T], BF16, name="resid0"),
                 x1pool.tile([128, DT, T], BF16, name="resid1")]

        # ================= Phase A: conv patch embedding =================
        with tc.tile_pool(name="conv", bufs=1) as convp, \
             tc.tile_pool(name="cps", bufs=3, space="PSUM") as cps, \
             tc.tile_pool(name="emb", bufs=2) as embp:
            wck_sb = convp.tile([128, DT, D], BF16)
            nc.sync.dma_start(wck_sb[:], wck.rearrange("(t p) d -> p t d", p=128))
            cb_sb = convp.tile([128, DT], F32)
            nc.sync.dma_start(cb_sb[:], cb.rearrange("(t p) -> p t", p=128))
            xp_sb = convp.tile([128, NI, DT, S], BF16)
            nc.sync.dma_start(xp_sb[:], xp.rearrange("b (t p) s -> p b t s", p=128))
            for img in range(NI):
                emb_sb = embp.tile([128, DT, S], BF16)
                for dm in range(DT):
                    for ch in range(QCH):
                        ps = cps.tile([128, QW], F32)
                        for kt in range(DT):
                            nc.tensor.matmul(
                                ps[:],
                                wck_sb[:, kt, dm * 128:(dm + 1) * 128],
                                xp_sb[:, img, kt, ch * QW:(ch + 1) * QW],
                                start=(kt == 0), stop=(kt == DT - 1))
                        nc.scalar.activation(
                            emb_sb[:, dm, ch * QW:(ch + 1) * QW], ps[:],
                            AF.Tanh, bias=cb_sb[:, dm:dm + 1])
                # write d-major flat: flat[(d, s)] with d = t*128 + p
                nc.sync.dma_start(
                    scratch[img].rearrange("s d -> (s d)").rearrange(
                        "(t p s) -> p t s", p=128, s=S),
                    emb_sb[:])

        # ============ Phase B: reshape quirk + pos-enc -> resid[0] ============
        with tc.tile_pool(name="htok", bufs=3) as hp, \
             tc.tile_pool(name="tps", bufs=4, space="PSUM") as tps, \
             tc.tile_pool(name="pe", bufs=1) as pep:
            pe_sb = pep.tile([128, DT, S], F32)
            nc.sync.dma_start(pe_sb[:], pef.rearrange("(t p) s -> p t s", p=128))
            for img in range(NI):
                for st in range(ST):
                    ss = min(128, S - st * 128)
                    h_sb = hp.tile([128, D], BF16)
                    nc.sync.dma_start(h_sb[:ss, :],
                                      scratch[img, st * 128:st * 128 + ss, :])
                    for dtile in range(DT):
                        pst = tps.tile([128, 128], BF16)
                        nc.tensor.transpose(
                            pst[:, 0:ss], h_sb[:ss, dtile * 128:(dtile + 1) * 128],
                            ident[0:ss, 0:ss])
                        nc.vector.tensor_add(
                            resid[0][:, dtile,
                                     img * S + st * 128: img * S + st * 128 + ss],
                            pst[:, 0:ss], pe_sb[:, dtile, st * 128:st * 128 + ss])

        tc.strict_bb_all_engine_barrier()

        # ---------------- layer-phase persistent pools ----------------
        wqkv = ctx.enter_context(tc.tile_pool(name="wqkv", bufs=4))
        qkp = ctx.enter_context(tc.tile_pool(name="qk", bufs=2))
        vp = ctx.enter_context(tc.tile_pool(name="v", bufs=2))
        ep = ctx.enter_context(tc.tile_pool(name="E", bufs=6))
        hvp = ctx.enter_context(tc.tile_pool(name="hv", bufs=1))
        xnp = ctx.enter_context(tc.tile_pool(name="xn", bufs=1))
        smallp = ctx.enter_context(tc.tile_pool(name="small", bufs=1))
        rsbp = ctx.enter_context(tc.tile_pool(name="rsb", bufs=3))
        ffw = ctx.enter_context(tc.tile_pool(name="ffw", bufs=4))
        gp = ctx.enter_context(tc.tile_pool(name="g", bufs=2))
        tmpp = ctx.enter_context(tc.tile_pool(name="tmp", bufs=2))

        # mutable pipeline state
        wsb = {}     # layer -> dict of weight tiles
        bsb = {}     # layer -> dict of bias tiles
        qk_t = {}    # halfstep -> qk tile
        v_t = {}     # halfstep -> v tile
        e_t = {}     # (halfstep, group) -> E tile [128, 2, ST, QW]

        def lay(k):
            return k // NI

        def im(k):
            return k % NI

        def rin(k):     # residual input of halfstep k's layer
            return resid[lay(k) % 2]

        def rout(k):    # residual output (x1 / final) of halfstep k's layer
            return resid[(lay(k) + 1) % 2]

        def emit_weights(li):
            w = {}
            for nm, dr in (("wq", wq), ("wk", wk), ("wv", wv), ("wh", wh)):
                t = wqkv.tile([128, DT, D], BF16, tag="w4", name=nm)
                nc.sync.dma_start(t[:], dr[li].rearrange("(t p) e -> p t e", p=128))
                w[nm] = t
            wsb[li] = w
            bb = {}
            for nm, dr, wd in (("whb", whb, DT), ("l2s", ln2s, DT),
                               ("l2b", ln2b, DT), ("b1", b1, FT), ("b2", b2, DT)):
                t = biasp.tile([128, wd], F32, tag=nm, name=nm)
                nc.sync.dma_start(t[:], dr[li].rearrange("(t p) -> p t", p=128))
                bb[nm] = t
            bsb[li] = bb

        def emit_c1c2(k, qps):
            """Q,K (feature-major) + V (token-major) projections for halfstep k.
            Yields between psum groups for interleaving."""
            li, img = lay(k), im(k)
            w = wsb[li]
            x = rin(k)
            qk_i = qkp.tile([128, 2 * DT, S], BF16, tag="qk", name="qk_i")
            qk_t[k] = qk_i
            v_i = vp.tile([128, ST, D], BF16, tag="v", name="v_i")
            v_t[k] = v_i
            for mi, w_sb in ((0, w["wq"]), (1, w["wk"])):
                for mt in range(DT):
                    for ch in range(QCH):
                        ps = qps.tile([128, 384], F32, name="qkps", tag="q")
                        for kt in range(DT):
                            nc.tensor.matmul(
                                ps[:, :QW], w_sb[:, kt, mt * 128:(mt + 1) * 128],
                                x[:, kt, img * S + ch * QW: img * S + (ch + 1) * QW],
                                start=(kt == 0), stop=(kt == DT - 1))
                        eng = nc.vector if (mt + ch) % 2 == 0 else nc.scalar
                        if eng is nc.vector:
                            nc.vector.tensor_copy(
                                qk_i[:, mi * DT + mt, ch * QW:(ch + 1) * QW],
                                ps[:, :QW])
                        else:
                            nc.scalar.activation(
                                qk_i[:, mi * DT + mt, ch * QW:(ch + 1) * QW],
                                ps[:, :QW], AF.Copy)
                    yield
            for (kt, row0, ss) in _stiles():
                for ch2 in range(2):
                    ps = qps.tile([128, 384], F32, name="vps", tag="q")
                    for dti in range(DT):
                        nc.tensor.matmul(
                            ps[:ss, :],
                            x[:, dti, img * S + row0: img * S + row0 + ss],
                            w["wv"][:, dti, ch2 * 384:(ch2 + 1) * 384],
                            start=(dti == 0), stop=(dti == DT - 1))
                    nc.vector.tensor_copy(
                        v_i[:ss, kt, ch2 * 384:(ch2 + 1) * 384], ps[:ss, :])
                yield

        def emit_scores(k, g, pool, tag="s"):
            """Scores + exp for group g = (head-pair, q-chunk) of halfstep k.
            Yields after each (h01, kt) matmul for fine interleaving."""
            hp_i, qc = g // QCH, g % QCH
            qk_i = qk_t[k]
            e = ep.tile([128, 2, ST, QW], BF16, tag="E", name="E_g")
            e_t[(k, g)] = e
            for h01 in range(2):
                for (kt, row0, ss) in _stiles():
                    ps = pool.tile([128, QW], F32, name="scps", tag=tag)
                    nc.tensor.matmul(
                        ps[0:ss, :],
                        qk_i[h01 * 64:(h01 + 1) * 64, DT + hp_i,
                             kt * 128:kt * 128 + ss],
                        qk_i[h01 * 64:(h01 + 1) * 64, hp_i,
                             qc * QW:(qc + 1) * QW],
                        start=True, stop=True)
                    nc.scalar.activation(e[0:ss, h01, kt, :], ps[0:ss, :],
                                         AF.Exp, scale=0.125)
                    yield

        def emit_hvd(k, g, hps, dps):
            """AV + denominator + normalize for group g of halfstep k.
            Yields after each (h01, kt) matmul pair for fine interleaving."""
            hp_i, qc = g // QCH, g % QCH
            v_i = v_t[k]
            e = e_t.pop((k, g))
            hv_i = hv_t[k]
            hv_ps = hps.tile([128, QW], F32, name="hvps", tag="h")
            d_ps = dps.tile([128, QW], F32, name="dps", tag="d")
            for h01 in range(2):
                for (kt, row0, ss) in _stiles():
                    nc.tensor.matmul(
                        hv_ps[h01 * 64:(h01 + 1) * 64, :],
                        v_i[0:ss, kt,
                            (2 * hp_i + h01) * 64:(2 * hp_i + h01 + 1) * 64],
                        e[0:ss, h01, kt, :],
                        start=(kt == 0), stop=(kt == ST - 1),
                        tile_position=(0, 64 * h01))
                    nc.tensor.matmul(
                        d_ps[h01 * 64:(h01 + 1) * 64, :],
                        ones64[0:ss, :],
                        e[0:ss, h01, kt, :],
                        start=(kt == 0), stop=(kt == ST - 1),
                        tile_position=(0, 64 * h01))
                    yield
            r_sb = rsbp.tile([128, QW], F32, tag="rsb")
            nc.vector.reciprocal_approx_fast(r_sb[:], d_ps[:])
            nc.vector.tensor_mul(
                hv_i[:, hp_i, qc * QW:(qc + 1) * QW], hv_ps[:], r_sb[:])

        def emit_c4(k, pool):
            """Wh projection + bias + residual -> rout (bf16)."""
            li, img = lay(k), im(k)
            hv_i = hv_t[k]
            x, x1 = rin(k), rout(k)
            w = wsb[li]["wh"]
            bb = bsb[li]
            for mt in range(DT):
                for ch in range(QCH):
                    ps = pool.tile([128, QW], F32, name="ops", tag="s")
                    for et in range(DT):
                        nc.tensor.matmul(
                            ps[:], w[:, et, mt * 128:(mt + 1) * 128],
                            hv_i[:, et, ch * QW:(ch + 1) * QW],
                            start=(et == 0), stop=(et == DT - 1))
                    nc.vector.scalar_tensor_tensor(
                        x1[:, mt, img * S + ch * QW: img * S + (ch + 1) * QW],
                        ps[:], bb["whb"][:, mt:mt + 1],
                        x[:, mt, img * S + ch * QW: img * S + (ch + 1) * QW],
                        op0=ADD, op1=ADD)
                yield

        def emit_ln(k, c, stps, bps, xn):
            """LayerNorm of rout cols [img*S + c*QW, +QW) -> xn chunk."""
            li, img = lay(k), im(k)
            x1 = rout(k)
            bb = bsb[li]
            c0 = img * S + c * QW
            sq = tmpp.tile([128, DT, QW], BF16, tag="sq")
            for kt in range(DT):
                nc.vector.tensor_mul(sq[:, kt, :], x1[:, kt, c0:c0 + QW],
                                     x1[:, kt, c0:c0 + QW])
            st = stps.tile([65, QW], F32, tag="st")
            for kt in range(DT):
                nc.tensor.matmul(st[0:1, :], ones_col[:], x1[:, kt, c0:c0 + QW],
                                 start=(kt == 0), stop=(kt == DT - 1))
                nc.tensor.matmul(st[64:65, :], ones_col[:], sq[:, kt, :],
                                 start=(kt == 0), stop=(kt == DT - 1))
            mom = smallp.tile([1, QW], F32, tag="mom")
            nc.scalar.mul(mom[:], st[0:1, :], 1.0 / D)
            msq = smallp.tile([1, QW], F32, tag="msq")
            nc.vector.tensor_mul(msq[:], mom[:], mom[:])
            ex2 = smallp.tile([1, QW], F32, tag="ex2")
            nc.scalar.mul(ex2[:], st[64:65, :], 1.0 / D)
            var = smallp.tile([1, QW], F32, tag="var")
            nc.vector.tensor_sub(var[:], ex2[:], msq[:])
            nc.scalar.activation(var[:], var[:], AF.Sqrt, bias=eps2[:])
            rstd = smallp.tile([1, QW], F32, tag="rstd")
            nc.vector.reciprocal_approx_fast(rstd[:], var[:])
            m_ps = bps.tile([128, QW], F32, tag="bc", name="m_ps")
            nc.tensor.matmul(m_ps[:], ones_row[:], mom[:], start=True, stop=True)
            r_ps = bps.tile([128, QW], F32, tag="bc", name="r_ps")
            nc.tensor.matmul(r_ps[:], ones_row[:], rstd[:], start=True, stop=True)
            for mt in range(DT):
                t_c = tmpp.tile([128, QW], F32, tag="t5a")
                nc.vector.tensor_sub(t_c[:], x1[:, mt, c0:c0 + QW], m_ps[:])
                t_d = tmpp.tile([128, QW], F32, tag="t5b")
                nc.vector.tensor_mul(t_d[:], t_c[:], r_ps[:])
                nc.scalar.activation(
                    xn[:, mt, c * QW:(c + 1) * QW], t_d[:],
                    AF.Identity, bias=bb["l2b"][:, mt:mt + 1],
                    scale=bb["l2s"][:, mt:mt + 1])

        def emit_ffn(k, c, f2p, gps, xn):
            """FFN over chunk c (QW cols) of halfstep k, += into rout.
            FFN2 accumulation lags FFN1 by one ft so the single g psum
            buffer never stalls the PE on gelu. Yields after each ft."""
            li, img = lay(k), im(k)
            x1 = rout(k)
            bb = bsb[li]
            c0 = img * S + c * QW
            f2 = [f2p.tile([128, QW], F32, tag=f"f2_{mt}", name=f"f2_{mt}")
                  for mt in range(DT)]
            prev = None

            def f2_accum(ft, g_bf, w2_sb):
                for mt in range(DT):
                    nc.tensor.matmul(
                        f2[mt][:], w2_sb[:, mt * 128:(mt + 1) * 128],
                        g_bf[:], start=(ft == 0), stop=(ft == FT - 1))

            for ft in range(FT):
                w1_sb = ffw.tile([128, DT, 128], BF16, tag="w1")
                nc.sync.dma_start(w1_sb[:], w1[li, ft])
                w2_sb = ffw.tile([128, D], BF16, tag="w2")
                nc.sync.dma_start(w2_sb[:], w2[li, ft * 128:(ft + 1) * 128, :])
                g_ps = gps.tile([128, QW], F32, tag="g")
                for kt in range(DT):
                    nc.tensor.matmul(
                        g_ps[:], w1_sb[:, kt, :], xn[:, kt, c * QW:(c + 1) * QW],
                        start=(kt == 0), stop=(kt == DT - 1))
                g_bf = gp.tile([128, QW], BF16, tag="gbf")
                nc.scalar.activation(g_bf[:], g_ps[:], AF.Gelu,
                                     bias=bb["b1"][:, ft:ft + 1])
                yield
                if prev is not None:
                    f2_accum(*prev)
                prev = (ft, g_bf, w2_sb)
            f2_accum(*prev)
            for mt in range(DT):
                nc.vector.scalar_tensor_tensor(
                    x1[:, mt, c0:c0 + QW], f2[mt][:], bb["b2"][:, mt:mt + 1],
                    x1[:, mt, c0:c0 + QW], op0=ADD, op1=ADD)

        def drain(gen):
            for _ in gen:
                pass

        hv_t = {}

        # ================= Phase C: pipelined encoder =================
        emit_weights(0)
        with tc.tile_pool(name="qps0", bufs=2, space="PSUM") as qps0:
            drain(emit_c1c2(0, qps0))
        with tc.tile_pool(name="scB0", bufs=2, space="PSUM") as scB0:
            for g in range(4):
                drain(emit_scores(0, g, scB0))

        for k in range(nk):
            li, img = lay(k), im(k)
            hv_t[k] = hvp.tile([128, DT, S], BF16, tag="hv", name="hv_i")

            # ---- B1: attention main + next-halfstep projections ----
            with tc.tile_pool(name="scps", bufs=3, space="PSUM") as scps, \
                 tc.tile_pool(name="qps", bufs=2, space="PSUM") as qps, \
                 tc.tile_pool(name="hps", bufs=2, space="PSUM") as hps, \
                 tc.tile_pool(name="dps", bufs=1, space="PSUM") as dps:
                if img == 1 and li + 1 < n_layers:
                    emit_weights(li + 1)
                c1c2 = emit_c1c2(k + 1, qps) if k + 1 < nk else iter(())
                for g in range(NG):
                    sc = (emit_scores(k, g + 4, scps) if g + 4 < NG
                          else iter(()))
                    hvd = emit_hvd(k, g, hps, dps)
                    while True:
                        a = next(sc, StopIteration)
                        b = next(hvd, StopIteration)
                        if a is StopIteration and b is StopIteration:
                            break
                    for _ in range(3):
                        next(c1c2, None)
                for _ in emit_c4(k, scps):
                    next(c1c2, None)
                drain(c1c2)

            xn = xnp.tile([128, DT, S], BF16, tag="xn", name="xn")
            # ---- B2 + B3: LN + FFN, overlapped with next-halfstep scores ----
            with tc.tile_pool(name="gsc", bufs=2, space="PSUM") as gsc:
                nxt_sc = []
                if k + 1 < nk:
                    nxt_sc = [emit_scores(k + 1, g, gsc, tag="g")
                              for g in range(4)]
                with tc.tile_pool(name="stps", bufs=1, space="PSUM") as stps, \
                     tc.tile_pool(name="bps", bufs=2, space="PSUM") as bps:
                    emit_ln(k, 0, stps, bps, xn)
                    if nxt_sc:
                        drain(nxt_sc[0])
                    emit_ln(k, 1, stps, bps, xn)
                    if nxt_sc:
                        drain(nxt_sc[1])
                with tc.tile_pool(name="f2ps", bufs=1, space="PSUM") as f2p:
                    for c in range(QCH):
                        sc = nxt_sc[2 + c] if len(nxt_sc) > 2 + c else iter(())
                        for _ in emit_ffn(k, c, f2p, gsc, xn):
                            next(sc, None)
                        drain(sc)

        # ================= Final LayerNorm -> out =================
        fin = resid[n_layers % 2]
        lnf_s = biasp.tile([128, DT], F32, tag="lnfs")
        nc.sync.dma_start(lnf_s[:], lnfs.rearrange("(t p) -> p t", p=128))
        lnf_b = biasp.tile([128, DT], F32, tag="lnfb")
        nc.sync.dma_start(lnf_b[:], lnfb.rearrange("(t p) -> p t", p=128))
        with tc.tile_pool(name="fout", bufs=2) as foutp, \
             tc.tile_pool(name="fstps", bufs=1, space="PSUM") as stps, \
             tc.tile_pool(name="fbps", bufs=2, space="PSUM") as bps:
            for ch in range(T // 384):
                sqf = tmpp.tile([128, DT, 384], BF16, tag="sq")
                for kt in range(DT):
                    nc.vector.tensor_mul(sqf[:, kt, :],
                                         fin[:, kt, ch * 384:(ch + 1) * 384],
                                         fin[:, kt, ch * 384:(ch + 1) * 384])
                st = stps.tile([65, 384], F32, tag="st")
                for kt in range(DT):
                    nc.tensor.matmul(st[0:1, :], ones_col[:],
                                     fin[:, kt, ch * 384:(ch + 1) * 384],
                                     start=(kt == 0), stop=(kt == DT - 1))
                    nc.tensor.matmul(st[64:65, :], ones_col[:], sqf[:, kt, :],
                                     start=(kt == 0), stop=(kt == DT - 1))
                mom = smallp.tile([1, 384], F32, tag="fmom")
                nc.scalar.mul(mom[:], st[0:1, :], 1.0 / D)
                msq = smallp.tile([1, 384], F32, tag="fmsq")
                nc.vector.tensor_mul(msq[:], mom[:], mom[:])
                ex2 = smallp.tile([1, 384], F32, tag="fex2")
                nc.scalar.mul(ex2[:], st[64:65, :], 1.0 / D)
                var = smallp.tile([1, 384], F32, tag="fvar")
                nc.vector.tensor_sub(var[:], ex2[:], msq[:])
                nc.scalar.activation(var[:], var[:], AF.Sqrt, bias=epsf[:])
                rstd = smallp.tile([1, 384], F32, tag="frstd")
                nc.vector.reciprocal_approx_fast(rstd[:], var[:])
                m_ps = bps.tile([128, 384], F32, tag="fbc", name="m_ps")
                nc.tensor.matmul(m_ps[:], ones_row[:], mom[:], start=True, stop=True)
                r_ps = bps.tile([128, 384], F32, tag="fbc", name="r_ps")
                nc.tensor.matmul(r_ps[:], ones_row[:], rstd[:], start=True, stop=True)
                for mt in range(DT):
                    t_c = tmpp.tile([128, 384], F32, tag="t5a")
                    nc.vector.tensor_sub(t_c[:],
                                         fin[:, mt, ch * 384:(ch + 1) * 384], m_ps[:])
                    t_d = tmpp.tile([128, 384], F32, tag="t5b")
                    nc.vector.tensor_mul(t_d[:], t_c[:], r_ps[:])
                    o_sb = foutp.tile([128, 384], F32)
                    nc.scalar.activation(o_sb[:], t_d[:], AF.Identity,
                                         bias=lnf_b[:, mt:mt + 1],
                                         scale=lnf_s[:, mt:mt + 1])
                    c0 = ch * 384
                    for off in range(0, 384, 192):
                        col = c0 + off
                        img, s0 = divmod(col, S)
                        nc.sync.dma_start(
                            out[img, mt * 128:(mt + 1) * 128, s0:s0 + 192],
                            o_sb[:, off:off + 192])
    nc.finalize()
    return nc


def _pos_encoding(max_len, d):
    pos = np.arange(max_len)[:, None].astype(np.float32)
    div = np.exp(np.arange(0, d, 2).astype(np.float32) * (-np.log(10000.0) / d))
    pe = np.zeros((max_len, d), dtype=np.float32)
    pe[:, 0::2] = np.sin(pos * div)
    pe[:, 1::2] = np.cos(pos * div)
    return pe


_NC_CACHE = {}


def get_nc(n_layers=L):
    if n_layers not in _NC_CACHE:
        _NC_CACHE[n_layers] = build_kernel(n_layers)
    return _NC_CACHE[n_layers]


def make_in_maps(x, conv_w, conv_b, ln1_s, ln1_b, wq, wk, wv, wh, wh_b,
                 ln2_s, ln2_b, w1, b1, w2, b2, lnf_s, lnf_b):
    bf = ml_dtypes.bfloat16
    x = np.asarray(x, np.float32)
    patches = x.reshape(B, C, IMG // P, P, IMG // P, P)      # (B,C,ty,py,tx,px)
    patches = patches.transpose(0, 1, 3, 5, 2, 4).reshape(B, D, S).astype(bf)
    wckh = np.ascontiguousarray(
        np.asarray(conv_w, np.float32).reshape(D, D).T).astype(bf)
    pefh = np.ascontiguousarray(_pos_encoding(5000, D)[:S].T)
    shared = {
        "wck": wckh, "cb": np.asarray(conv_b, np.float32), "pef": pefh,
        "wq": np.asarray(wq, np.float32).astype(bf),
        "wk": np.asarray(wk, np.float32).astype(bf),
        "wv": np.asarray(wv, np.float32).astype(bf),
        "wh": np.asarray(wh, np.float32).astype(bf),
        "whb": np.asarray(wh_b, np.float32),
        "ln2s": np.asarray(ln2_s, np.float32),
        "ln2b": np.asarray(ln2_b, np.float32),
        "w1": np.ascontiguousarray(
            np.asarray(w1, np.float32).reshape(L, DT, 128, FT, 128)
            .transpose(0, 3, 2, 1, 4)).astype(bf),
        "b1": np.asarray(b1, np.float32),
        "w2": np.asarray(w2, np.float32).astype(bf),
        "b2": np.asarray(b2, np.float32),
        "lnfs": np.asarray(lnf_s, np.float32),
        "lnfb": np.asarray(lnf_b, np.float32),
    }
    in_maps = []
    for c in range(NCORES):
        m = dict(shared)
        m["xp"] = np.ascontiguousarray(patches[c * NI:(c + 1) * NI])
        in_maps.append(m)
    return in_maps


def assemble_output(results):
    out = np.empty((B, S, D), np.float32)
    for c in range(NCORES):
        o = results[c]["out"]
        for i in range(NI):
            out[c * NI + i] = o[i].T
    return out


def kernel(**inputs) -> np.ndarray:
    nc = get_nc()
    in_maps = make_in_maps(**inputs)
    res = run_bass_kernel_spmd(nc, in_maps, core_ids=list(range(NCORES)))
    return assemble_output(res.results)


# revision 27
# speedup vs baseline: 1.1667x; 1.1667x over previous
"""Trainium2 Bass kernel for nn_Encoder_38259568672815 (ViT-style encoder).

Strategy: data-parallel over batch (16 images -> 8 cores x 2 images).
On-chip layout: feature-major residual stream [D on partitions, tokens free]
in bf16. Software-pipelined over 12 halfsteps (layer, image): attention of
halfstep k+1 (scores+exp on Scalar) overlaps the FFN of halfstep k (PE), so
the Tensor engine stays dense and at full clock.

Self-contained: hardcodes all shapes; host work is limited to layout
permutations (im2col, weight casts, final transpose) and sharding.
"""
from contextlib import ExitStack

import numpy as np
import ml_dtypes

import concourse.bass as bass
import concourse.tile as tile
import concourse.mybir as mybir
from concourse import bacc
from concourse.masks import make_identity
from concourse.bass_utils import run_bass_kernel_spmd

F32 = mybir.dt.float32
BF16 = mybir.dt.bfloat16
AF = mybir.ActivationFunctionType
ADD = mybir.AluOpType.add

B, C, IMG, P = 16, 3, 384, 16
D, NH, DK, L, FF = 768, 12, 64, 6, 3072
S = (IMG // P) ** 2          # 576 tokens per image
NI = 2                       # images per core
T = NI * S                   # 1152 token columns per core
DT = D // 128                # 6 d-tiles
FT = FF // 128               # 24 f-tiles
ST = (S + 127) // 128        # 5 token tiles per image (last = 64)
QCH = 2                      # q chunks of 288 per image
QW = S // QCH                # 288
NG = NH // 2 * QCH           # 12 (head-pair, q-chunk) groups per halfstep
NK = NI * L                  # 12 halfsteps
NCORES = 8


def _stiles():
    return [(kt, kt * 128, min(128, S - kt * 128)) for kt in range(ST)]


def build_kernel(n_layers=L):
    nc = bacc.Bacc()
    nk = NI * n_layers

    xp = nc.dram_tensor("xp", [NI, D, S], BF16, kind="ExternalInput")
    wck = nc.dram_tensor("wck", [D, D], BF16, kind="ExternalInput")
    cb = nc.dram_tensor("cb", [D], F32, kind="ExternalInput")
    pef = nc.dram_tensor("pef", [D, S], F32, kind="ExternalInput")
    wq = nc.dram_tensor("wq", [L, D, D], BF16, kind="ExternalInput")
    wk = nc.dram_tensor("wk", [L, D, D], BF16, kind="ExternalInput")
    wv = nc.dram_tensor("wv", [L, D, D], BF16, kind="ExternalInput")
    wh = nc.dram_tensor("wh", [L, D, D], BF16, kind="ExternalInput")
    whb = nc.dram_tensor("whb", [L, D], F32, kind="ExternalInput")
    ln2s = nc.dram_tensor("ln2s", [L, D], F32, kind="ExternalInput")
    ln2b = nc.dram_tensor("ln2b", [L, D], F32, kind="ExternalInput")
    w1 = nc.dram_tensor("w1", [L, FT, 128, DT, 128], BF16, kind="ExternalInput")
    b1 = nc.dram_tensor("b1", [L, FF], F32, kind="ExternalInput")
    w2 = nc.dram_tensor("w2", [L, FF, D], BF16, kind="ExternalInput")
    b2 = nc.dram_tensor("b2", [L, D], F32, kind="ExternalInput")
    lnfs = nc.dram_tensor("lnfs", [D], F32, kind="ExternalInput")
    lnfb = nc.dram_tensor("lnfb", [D], F32, kind="ExternalInput")
    out = nc.dram_tensor("out", [NI, D, S], F32, kind="ExternalOutput")
    scratch = nc.dram_tensor("scratch", [NI, S, D], BF16)

    with tile.TileContext(nc) as tc, ExitStack() as ctx, \
            nc.allow_low_precision(reason="bf16 residual stream by design"):
        xpool = ctx.enter_context(tc.tile_pool(name="x", bufs=1))
        x1pool = ctx.enter_context(tc.tile_pool(name="x1", bufs=1))
        consts = ctx.enter_context(tc.tile_pool(name="consts", bufs=1))
        biasp = ctx.enter_context(tc.tile_pool(name="biasp", bufs=2))

        ones_col = consts.tile([128, 1], BF16)
        nc.vector.memset(ones_col[:], 1.0)
        ones_f = consts.tile([128, 1], F32)
        nc.vector.memset(ones_f[:], 1.0)
        ones64 = consts.tile([128, 64], BF16)
        nc.vector.memset(ones64[:], 1.0)
        ones_row = consts.tile([1, 128], F32)
        nc.vector.memset(ones_row[:], 1.0)
        eps2 = consts.tile([1, 1], F32)
        nc.vector.memset(eps2[:], 1e-6)
        epsf = consts.tile([1, 1], F32)
        nc.vector.memset(epsf[:], 1e-12)
        ident = consts.tile([128, 128], BF16)
        make_identity(nc, ident[:])

        resid = [xpool.tile([128, DT, T], BF16, name="resid0"),
                 x1pool.tile([128, DT, T], BF16, name="resid1")]

        # ================= Phase A: conv patch embedding =================
        with tc.tile_pool(name="conv", bufs=1) as convp, \
             tc.tile_pool(name="cps", bufs=3, space="PSUM") as cps, \
             tc.tile_pool(name="emb", bufs=2) as embp:
            wck_sb = convp.tile([128, DT, D], BF16)
            nc.sync.dma_start(wck_sb[:], wck.rearrange("(t p) d -> p t d", p=128))
            cb_sb = convp.tile([128, DT], F32)
            nc.sync.dma_start(cb_sb[:], cb.rearrange("(t p) -> p t", p=128))
            xp_sb = convp.tile([128, NI, DT, S], BF16)
            nc.sync.dma_start(xp_sb[:], xp.rearrange("b (t p) s -> p b t s", p=128))
            for img in range(NI):
                emb_sb = embp.tile([128, DT, S], BF16)
                for dm in range(DT):
                    for ch in range(QCH):
                        ps = cps.tile([128, QW], F32)
                        for kt in range(DT):
                            nc.tensor.matmul(
                                ps[:],
                                wck_sb[:, kt, dm * 128:(dm + 1) * 128],
                                xp_sb[:, img, kt, ch * QW:(ch + 1) * QW],
                                start=(kt == 0), stop=(kt == DT - 1))
                        nc.scalar.activation(
                            emb_sb[:, dm, ch * QW:(ch + 1) * QW], ps[:],
                            AF.Tanh, bias=cb_sb[:, dm:dm + 1])
                # write d-major flat: flat[(d, s)] with d = t*128 + p
                nc.sync.dma_start(
                    scratch[img].rearrange("s d -> (s d)").rearrange(
                        "(t p s) -> p t s", p=128, s=S),
                    emb_sb[:])

        # ============ Phase B: reshape quirk + pos-enc -> resid[0] ============
        with tc.tile_pool(name="htok", bufs=3) as hp, \
             tc.tile_pool(name="tps", bufs=4, space="PSUM") as tps, \
             tc.tile_pool(name="pe", bufs=1) as pep:
            pe_sb = pep.tile([128, DT, S], F32)
            nc.sync.dma_start(pe_sb[:], pef.rearrange("(t p) s -> p t s", p=128))
            for img in range(NI):
                for st in range(ST):
                    ss = min(128, S - st * 128)
                    h_sb = hp.tile([128, D], BF16)
                    nc.sync.dma_start(h_sb[:ss, :],
                                      scratch[img, st * 128:st * 128 + ss, :])
                    for dtile in range(DT):
                        pst = tps.tile([128, 128], BF16)
                        nc.tensor.transpose(
                            pst[:, 0:ss], h_sb[:ss, dtile * 128:(dtile + 1) * 128],
                            ident[0:ss, 0:ss])
                        nc.vector.tensor_add(
                            resid[0][:, dtile,
                                     img * S + st * 128: img * S + st * 128 + ss],
                            pst[:, 0:ss], pe_sb[:, dtile, st * 128:st * 128 + ss])

        tc.strict_bb_all_engine_barrier()

        # ---------------- layer-phase persistent pools ----------------
        wqkv = ctx.enter_context(tc.tile_pool(name="wqkv", bufs=4))
        qkp = ctx.enter_context(tc.tile_pool(name="qk", bufs=2))
        vp = ctx.enter_context(tc.tile_pool(name="v", bufs=2))
        ep = ctx.enter_context(tc.tile_pool(name="E", bufs=6))
        hvp = ctx.enter_context(tc.tile_pool(name="hv", bufs=1))
        xnp = ctx.enter_context(tc.tile_pool(name="xn", bufs=1))
        smallp = ctx.enter_context(tc.tile_pool(name="small", bufs=1))
        rsbp = ctx.enter_context(tc.tile_pool(name="rsb", bufs=3))
        ffw = ctx.enter_context(tc.tile_pool(name="ffw", bufs=4))
        gp = ctx.enter_context(tc.tile_pool(name="g", bufs=2))
        tmpp = ctx.enter_context(tc.tile_pool(name="tmp", bufs=2))

        # mutable pipeline state
        wsb = {}     # layer -> dict of weight tiles
        bsb = {}     # layer -> dict of bias tiles
        qk_t = {}    # halfstep -> qk tile
        v_t = {}     # halfstep -> v tile
        e_t = {}     # (halfstep, group) -> E tile [128, 2, ST, QW]

        def lay(k):
            return k // NI

        def im(k):
            return k % NI

        def rin(k):     # residual input of halfstep k's layer
            return resid[lay(k) % 2]

        def rout(k):    # residual output (x1 / final) of halfstep k's layer
            return resid[(lay(k) + 1) % 2]

        def emit_weights(li):
            w = {}
            for nm, dr in (("wq", wq), ("wk", wk), ("wv", wv), ("wh", wh)):
                t = wqkv.tile([128, DT, D], BF16, tag="w4", name=nm)
                nc.sync.dma_start(t[:], dr[li].rearrange("(t p) e -> p t e", p=128))
                w[nm] = t
            wsb[li] = w
            bb = {}
            for nm, dr, wd in (("whb", whb, DT), ("l2s", ln2s, DT),
                               ("l2b", ln2b, DT), ("b1", b1, FT), ("b2", b2, DT)):
                t = biasp.tile([128, wd], F32, tag=nm, name=nm)
                nc.sync.dma_start(t[:], dr[li].rearrange("(t p) -> p t", p=128))
                bb[nm] = t
            bsb[li] = bb

        def emit_c1c2(k, qps):
            """Q,K (feature-major) + V (token-major) projections for halfstep k.
            Yields between psum groups for interleaving."""
            li, img = lay(k), im(k)
            w = wsb[li]
            x = rin(k)
            qk_i = qkp.tile([128, 2 * DT, S], BF16, tag="qk", name="qk_i")
            qk_t[k] = qk_i
            v_i = vp.tile([128, ST, D], BF16, tag="v", name="v_i")
            v_t[k] = v_i
            for mi, w_sb in ((0, w["wq"]), (1, w["wk"])):
                for mt in range(DT):
                    for ch in range(QCH):
                        ps = qps.tile([128, 384], F32, name="qkps", tag="q")
                        for kt in range(DT):
                            nc.tensor.matmul(
                                ps[:, :QW], w_sb[:, kt, mt * 128:(mt + 1) * 128],
                                x[:, kt, img * S + ch * QW: img * S + (ch + 1) * QW],
                                start=(kt == 0), stop=(kt == DT - 1))
                        eng = nc.vector if (mt + ch) % 2 == 0 else nc.scalar
                        if eng is nc.vector:
                            nc.vector.tensor_copy(
                                qk_i[:, mi * DT + mt, ch * QW:(ch + 1) * QW],
                                ps[:, :QW])
                        else:
                            nc.scalar.activation(
                                qk_i[:, mi * DT + mt, ch * QW:(ch + 1) * QW],
                                ps[:, :QW], AF.Copy)
                    yield
            for (kt, row0, ss) in _stiles():
                for ch2 in range(2):
                    ps = qps.tile([128, 384], F32, name="vps", tag="q")
                    for dti in range(DT):
                        nc.tensor.matmul(
                            ps[:ss, :],
                            x[:, dti, img * S + row0: img * S + row0 + ss],
                            w["wv"][:, dti, ch2 * 384:(ch2 + 1) * 384],
                            start=(dti == 0), stop=(dti == DT - 1))
                    nc.vector.tensor_copy(
                        v_i[:ss, kt, ch2 * 384:(ch2 + 1) * 384], ps[:ss, :])
                yield

        def emit_scores(k, g, pool, tag="s"):
            """Scores + exp for group g = (head-pair, q-chunk) of halfstep k.
            Yields after each (h01, kt) matmul for fine interleaving."""
            hp_i, qc = g // QCH, g % QCH
            qk_i = qk_t[k]
            e = ep.tile([128, 2, ST, QW], BF16, tag="E", name="E_g")
            e_t[(k, g)] = e
            for h01 in range(2):
                for (kt, row0, ss) in _stiles():
                    ps = pool.tile([128, QW], F32, name="scps", tag=tag)
                    nc.tensor.matmul(
                        ps[0:ss, :],
                        qk_i[h01 * 64:(h01 + 1) * 64, DT + hp_i,
                             kt * 128:kt * 128 + ss],
                        qk_i[h01 * 64:(h01 + 1) * 64, hp_i,
                             qc * QW:(qc + 1) * QW],
                        start=True, stop=True)
                    nc.scalar.activation(e[0:ss, h01, kt, :], ps[0:ss, :],
                                         AF.Exp, scale=0.125)
                    yield

        def emit_hvd(k, g, hps, dps):
            """AV + denominator + normalize for group g of halfstep k.
            Yields after each (h01, kt) matmul pair for fine interleaving."""
            hp_i, qc = g // QCH, g % QCH
            v_i = v_t[k]
            e = e_t.pop((k, g))
            hv_i = hv_t[k]
            hv_ps = hps.tile([128, QW], F32, name="hvps", tag="h")
            d_ps = dps.tile([128, QW], F32, name="dps", tag="d")
            for h01 in range(2):
                for (kt, row0, ss) in _stiles():
                    nc.tensor.matmul(
                        hv_ps[h01 * 64:(h01 + 1) * 64, :],
                        v_i[0:ss, kt,
                            (2 * hp_i + h01) * 64:(2 * hp_i + h01 + 1) * 64],
                        e[0:ss, h01, kt, :],
                        start=(kt == 0), stop=(kt == ST - 1),
                        tile_position=(0, 64 * h01))
                    nc.tensor.matmul(
                        d_ps[h01 * 64:(h01 + 1) * 64, :],
                        ones64[0:ss, :],
                        e[0:ss, h01, kt, :],
                        start=(kt == 0), stop=(kt == ST - 1),
                        tile_position=(0, 64 * h01))
                    yield
            r_sb = rsbp.tile([128, QW], F32, tag="rsb")
            nc.vector.reciprocal_approx_fast(r_sb[:], d_ps[:])
            nc.vector.tensor_mul(
                hv_i[:, hp_i, qc * QW:(qc + 1) * QW], hv_ps[:], r_sb[:])

        def emit_c4(k, pool):
            """Wh projection + bias + residual -> rout (bf16)."""
            li, img = lay(k), im(k)
            hv_i = hv_t[k]
            x, x1 = rin(k), rout(k)
            w = wsb[li]["wh"]
            bb = bsb[li]
            for mt in range(DT):
                for ch in range(QCH):
                    ps = pool.tile([128, QW], F32, name="ops", tag="s")
                    for et in range(DT):
                        nc.tensor.matmul(
                            ps[:], w[:, et, mt * 128:(mt + 1) * 128],
                            hv_i[:, et, ch * QW:(ch + 1) * QW],
                            start=(et == 0), stop=(et == DT - 1))
                    nc.vector.scalar_tensor_tensor(
                        x1[:, mt, img * S + ch * QW: img * S + (ch + 1) * QW],
                        ps[:], bb["whb"][:, mt:mt + 1],
                        x[:, mt, img * S + ch * QW: img * S + (ch + 1) * QW],
                        op0=ADD, op1=ADD)
                yield

        def emit_ln(k, c, stps, bps, xn):
            """LayerNorm of rout cols [img*S + c*QW, +QW) -> xn chunk."""
            li, img = lay(k), im(k)
            x1 = rout(k)
            bb = bsb[li]
            c0 = img * S + c * QW
            sq = tmpp.tile([128, DT, QW], BF16, tag="sq")
            for kt in range(DT):
                nc.vector.tensor_mul(sq[:, kt, :], x1[:, kt, c0:c0 + QW],
                                     x1[:, kt, c0:c0 + QW])
            st = stps.tile([65, QW], F32, tag="st")
            for kt in range(DT):
                nc.tensor.matmul(st[0:1, :], ones_col[:], x1[:, kt, c0:c0 + QW],
                                 start=(kt == 0), stop=(kt == DT - 1))
                nc.tensor.matmul(st[64:65, :], ones_col[:], sq[:, kt, :],
                                 start=(kt == 0), stop=(kt == DT - 1))
            mom = smallp.tile([1, QW], F32, tag="mom")
            nc.scalar.mul(mom[:], st[0:1, :], 1.0 / D)
            msq = smallp.tile([1, QW], F32, tag="msq")
            nc.vector.tensor_mul(msq[:], mom[:], mom[:])
            ex2 = smallp.tile([1, QW], F32, tag="ex2")
            nc.scalar.mul(ex2[:], st[64:65, :], 1.0 / D)
            var = smallp.tile([1, QW], F32, tag="var")
            nc.vector.tensor_sub(var[:], ex2[:], msq[:])
            nc.scalar.activation(var[:], var[:], AF.Sqrt, bias=eps2[:])
            rstd = smallp.tile([1, QW], F32, tag="rstd")
            nc.vector.reciprocal_approx_fast(rstd[:], var[:])
            m_ps = bps.tile([128, QW], F32, tag="bc", name="m_ps")
            nc.tensor.matmul(m_ps[:], ones_row[:], mom[:], start=True, stop=True)
            r_ps = bps.tile([128, QW], F32, tag="bc", name="r_ps")
            nc.tensor.matmul(r_ps[:], ones_row[:], rstd[:], start=True, stop=True)
            for mt in range(DT):
                t_c = tmpp.tile([128, QW], F32, tag="t5a")
                nc.vector.tensor_sub(t_c[:], x1[:, mt, c0:c0 + QW], m_ps[:])
                t_d = tmpp.tile([128, QW], F32, tag="t5b")
                nc.vector.tensor_mul(t_d[:], t_c[:], r_ps[:])
                nc.scalar.activation(
                    xn[:, mt, c * QW:(c + 1) * QW], t_d[:],
                    AF.Identity, bias=bb["l2b"][:, mt:mt + 1],
                    scale=bb["l2s"][:, mt:mt + 1])

        def emit_ffn(k, c, f2p, gps, xn):
            """FFN over chunk c (QW cols) of halfstep k, += into rout.
            FFN2 accumulation lags FFN1 by one ft so the single g psum
            buffer never stalls the PE on gelu. Yields after each ft."""
            li, img = lay(k), im(k)
            x1 = rout(k)
            bb = bsb[li]
            c0 = img * S + c * QW
            f2 = [f2p.tile([128, QW], F32, tag=f"f2_{mt}", name=f"f2_{mt}")
                  for mt in range(DT)]
            prev = None

            def f2_accum(ft, g_bf, w2_sb):
                for mt in range(DT):
                    nc.tensor.matmul(
                        f2[mt][:], w2_sb[:, mt * 128:(mt + 1) * 128],
                        g_bf[:], start=(ft == 0), stop=(ft == FT - 1))

            for ft in range(FT):
                w1_sb = ffw.tile([128, DT, 128], BF16, tag="w1")
                nc.sync.dma_start(w1_sb[:], w1[li, ft])
                w2_sb = ffw.tile([128, D], BF16, tag="w2")
                nc.sync.dma_start(w2_sb[:], w2[li, ft * 128:(ft + 1) * 128, :])
                g_ps = gps.tile([128, QW], F32, tag="g")
                for kt in range(DT):
                    nc.tensor.matmul(
                        g_ps[:], w1_sb[:, kt, :], xn[:, kt, c * QW:(c + 1) * QW],
                        start=(kt == 0), stop=(kt == DT - 1))
                g_bf = gp.tile([128, QW], BF16, tag="gbf")
                nc.scalar.activation(g_bf[:], g_ps[:], AF.Gelu,
                                     bias=bb["b1"][:, ft:ft + 1])
                yield
                if prev is not None:
                    f2_accum(*prev)
                prev = (ft, g_bf, w2_sb)
            f2_accum(*prev)
            for mt in range(DT):
                nc.vector.scalar_tensor_tensor(
                    x1[:, mt, c0:c0 + QW], f2[mt][:], bb["b2"][:, mt:mt + 1],
                    x1[:, mt, c0:c0 + QW], op0=ADD, op1=ADD)

        def drain(gen):
            for _ in gen:
                pass

        hv_t = {}

        # ================= Phase C: pipelined encoder =================
        emit_weights(0)
        with tc.tile_pool(name="qps0", bufs=2, space="PSUM") as qps0:
            drain(emit_c1c2(0, qps0))
        with tc.tile_pool(name="scB0", bufs=2, space="PSUM") as scB0:
            for g in range(4):
                drain(emit_scores(0, g, scB0))

        for k in range(nk):
            li, img = lay(k), im(k)
            hv_t[k] = hvp.tile([128, DT, S], BF16, tag="hv", name="hv_i")

            # ---- B1: attention main + next-halfstep projections/scores ----
            # All exp activations live here (scalar = exp-only), so the
            # act-table never thrashes against B3's gelu.
            with tc.tile_pool(name="scps", bufs=3, space="PSUM") as scps, \
                 tc.tile_pool(name="qps", bufs=2, space="PSUM") as qps, \
                 tc.tile_pool(name="hps", bufs=2, space="PSUM") as hps, \
                 tc.tile_pool(name="dps", bufs=1, space="PSUM") as dps:
                if img == 1 and li + 1 < n_layers:
                    emit_weights(li + 1)
                c1c2 = emit_c1c2(k + 1, qps) if k + 1 < nk else iter(())
                for g in range(NG):
                    if g + 4 < NG:
                        sc = emit_scores(k, g + 4, scps)
                    elif k + 1 < nk:
                        sc = emit_scores(k + 1, g + 4 - NG, scps)
                    else:
                        sc = iter(())
                    hvd = emit_hvd(k, g, hps, dps)
                    while True:
                        a = next(sc, StopIteration)
                        b = next(hvd, StopIteration)
                        if a is StopIteration and b is StopIteration:
                            break
                    for _ in range(3):
                        next(c1c2, None)
                for _ in emit_c4(k, scps):
                    next(c1c2, None)
                drain(c1c2)

            xn = xnp.tile([128, DT, S], BF16, tag="xn", name="xn")
            # ---- B2: LayerNorm ----
            with tc.tile_pool(name="stps", bufs=1, space="PSUM") as stps, \
                 tc.tile_pool(name="bps", bufs=2, space="PSUM") as bps:
                emit_ln(k, 0, stps, bps, xn)
                emit_ln(k, 1, stps, bps, xn)
            # ---- B3: FFN (PE-dense, scalar = gelu-only) ----
            with tc.tile_pool(name="f2ps", bufs=1, space="PSUM") as f2p, \
                 tc.tile_pool(name="gps", bufs=2, space="PSUM") as gps:
                for c in range(QCH):
                    drain(emit_ffn(k, c, f2p, gps, xn))

        # ================= Final LayerNorm -> out =================
        fin = resid[n_layers % 2]
        lnf_s = biasp.tile([128, DT], F32, tag="lnfs")
        nc.sync.dma_start(lnf_s[:], lnfs.rearrange("(t p) -> p t", p=128))
        lnf_b = biasp.tile([128, DT], F32, tag="lnfb")
        nc.sync.dma_start(lnf_b[:], lnfb.rearrange("(t p) -> p t", p=128))
        with tc.tile_pool(name="fout", bufs=2) as foutp, \
             tc.tile_pool(name="fstps", bufs=1, space="PSUM") as stps, \
             tc.tile_pool(name="fbps", bufs=2, space="PSUM") as bps:
            for ch in range(T // 384):
                sqf = tmpp.tile([128, DT, 384], BF16, tag="sq")
                for kt in range(DT):
                    nc.vector.tensor_mul(sqf[:, kt, :],
                                         fin[:, kt, ch * 384:(ch + 1) * 384],
                                         fin[:, kt, ch * 384:(ch + 1) * 384])
                st = stps.tile([65, 384], F32, tag="st")
                for kt in range(DT):
                    nc.tensor.matmul(st[0:1, :], ones_col[:],
                                     fin[:, kt, ch * 384:(ch + 1) * 384],
                                     start=(kt == 0), stop=(kt == DT - 1))
                    nc.tensor.matmul(st[64:65, :], ones_col[:], sqf[:, kt, :],
                                     start=(kt == 0), stop=(kt == DT - 1))
                mom = smallp.tile([1, 384], F32, tag="fmom")
                nc.scalar.mul(mom[:], st[0:1, :], 1.0 / D)
                msq = smallp.tile([1, 384], F32, tag="fmsq")
                nc.vector.tensor_mul(msq[:], mom[:], mom[:])
                ex2 = smallp.tile([1, 384], F32, tag="fex2")
                nc.scalar.mul(ex2[:], st[64:65, :], 1.0 / D)
                var = smallp.tile([1, 384], F32, tag="fvar")
                nc.vector.tensor_sub(var[:], ex2[:], msq[:])
                nc.scalar.activation(var[:], var[:], AF.Sqrt, bias=epsf[:])
                rstd = smallp.tile([1, 384], F32, tag="frstd")
                nc.vector.reciprocal_approx_fast(rstd[:], var[:])
                m_ps = bps.tile([128, 384], F32, tag="fbc", name="m_ps")
                nc.tensor.matmul(m_ps[:], ones_row[:], mom[:], start=True, stop=True)
                r_ps = bps.tile([128, 384], F32, tag="fbc", name="r_ps")
                nc.tensor.matmul(r_ps[:], ones_row[:], rstd[:], start=True, stop=True)
                for mt in range(DT):
                    t_c = tmpp.tile([128, 384], F32, tag="t5a")
                    nc.vector.tensor_sub(t_c[:],
                                         fin[:, mt, ch * 384:(ch + 1) * 384], m_ps[:])
                    t_d = tmpp.tile([128, 384], F32, tag="t5b")
                    nc.vector.tensor_mul(t_d[:], t_c[:], r_ps[:])
                    o_sb = foutp.tile([128, 384], F32)
                    nc.scalar.activation(o_sb[:], t_d[:], AF.Identity,
                                         bias=lnf_b[:, mt:mt + 1],
                                         scale=lnf_s[:, mt:mt + 1])
                    c0 = ch * 384
                    for off in range(0, 384, 192):
                        col = c0 + off
                        img, s0 = divmod(col, S)
                        nc.sync.dma_start(
                            out[img, mt * 128:(mt + 1) * 128, s0:s0 + 192],
                            o_sb[:, off:off + 192])
    nc.finalize()
    return nc


def _pos_encoding(max_len, d):
    pos = np.arange(max_len)[:, None].astype(np.float32)
    div = np.exp(np.arange(0, d, 2).astype(np.float32) * (-np.log(10000.0) / d))
    pe = np.zeros((max_len, d), dtype=np.float32)
    pe[:, 0::2] = np.sin(pos * div)
    pe[:, 1::2] = np.cos(pos * div)
    return pe


_NC_CACHE = {}


def get_nc(n_layers=L):
    if n_layers not in _NC_CACHE:
        _NC_CACHE[n_layers] = build_kernel(n_layers)
    return _NC_CACHE[n_layers]


def make_in_maps(x, conv_w, conv_b, ln1_s, ln1_b, wq, wk, wv, wh, wh_b,
                 ln2_s, ln2_b, w1, b1, w2, b2, lnf_s, lnf_b):
    bf = ml_dtypes.bfloat16
    x = np.asarray(x, np.float32)
    patches = x.reshape(B, C, IMG // P, P, IMG // P, P)      # (B,C,ty,py,tx,px)
    patches = patches.transpose(0, 1, 3, 5, 2, 4).reshape(B, D, S).astype(bf)
    wckh = np.ascontiguousarray(
        np.asarray(conv_w, np.float32).reshape(D, D).T).astype(bf)
    pefh = np.ascontiguousarray(_pos_encoding(5000, D)[:S].T)
    shared = {
        "wck": wckh, "cb": np.asarray(conv_b, np.float32), "pef": pefh,
        "wq": np.asarray(wq, np.float32).astype(bf),
        "wk": np.asarray(wk, np.float32).astype(bf),
        "wv": np.asarray(wv, np.float32).astype(bf),
        "wh": np.asarray(wh, np.float32).astype(bf),
        "whb": np.asarray(wh_b, np.float32),
        "ln2s": np.asarray(ln2_s, np.float32),
        "ln2b": np.asarray(ln2_b, np.float32),
        "w1": np.ascontiguousarray(
            np.asarray(w1, np.float32).reshape(L, DT, 128, FT, 128)
            .transpose(0, 3, 2, 1, 4)).astype(bf),
        "b1": np.asarray(b1, np.float32),
        "w2": np.asarray(w2, np.float32).astype(bf),
        "b2": np.asarray(b2, np.float32),
        "lnfs": np.asarray(lnf_s, np.float32),
        "lnfb": np.asarray(lnf_b, np.float32),
    }
    in_maps = []
    for c in range(NCORES):
        m = dict(shared)
        m["xp"] = np.ascontiguousarray(patches[c * NI:(c + 1) * NI])
        in_maps.append(m)
    return in_maps


def assemble_output(results):
    out = np.empty((B, S, D), np.float32)
    for c in range(NCORES):
        o = results[c]["out"]
        for i in range(NI):
            out[c * NI + i] = o[i].T
    return out


def kernel(**inputs) -> np.ndarray:
    nc = get_nc()
    in_maps = make_in_maps(**inputs)
    res = run_bass_kernel_spmd(nc, in_maps, core_ids=list(range(NCORES)))
    return assemble_output(res.results)
